# revision 38
# baseline (speedup 1.0000x reference)
"""Trainium2 Bass kernel for nn_BackBone (LSTM backbone + fc + outer-product head).

Data-parallel over batch across 8 NeuronCores. Per core (b_loc rows), v9:
  - history transposed + cast to fp16 on the HOST: xt[D+1, T, B] with a
    constant-1.0 feature row appended so the gate bias rides the projection
    matmul. All matmuls fp16 (fp8 DoubleRow measured 424ns per 512-col MM =
    exactly two fp16 matmuls; no streaming win, so fp16 keeps accuracy).
  - two 512-col batch chains with chain B OFFSET one step behind chain A,
    per-READER gate PSUMs (Pif merged sigmoid(i,f), Pg tanh, Po sigmoid) so
    proj(t+1) matmuls only wait on their own gate's activation read ->
    near-zero-stall PE steady state at ~215ns/MM (the binding engine:
    32 N=512 matmuls/step = 6.9us vs ACT 6.5us).
  - head einsum: broadcast tensor_tensor with PAIR-DUPLICATED pref
    (pf2[...,2]); the step-1 inner fp16 pair unlocks DVE 2x_1P mode: 1.15us
    per [128,5,3,128] job vs 2.15us classic. y2 jobs ride the recurrence on
    DVE + small GpSimd bites (no ACT einsum while ACT paces the LSTM; GpSimd
    and DVE share an SBUF port, so concurrent big jobs on both mutually
    stall - GpSimd gets only small early bites).
  - y1 tail: chain A finishes one step early; its h transposes run on the PE
    (three into distinct DEAD gate-PSUM banks via fp16 bitcast - distinct
    tiles because Tile WAR tracking is tile-granular - plus one DMA-xbar)
    and its 8 einsum jobs hide under chain B's final solo step. Only chain
    B's short tail (PE transposes + 7 DVE jobs + 1 ACT job) is exposed.
  - y2 head borrows chain B's Po/Pg PSUM banks between the two t=0
    projection groups; relu on DVE (tensor_scalar_max) keeps the single ACT
    table-set; a dummy sigmoid preloads the ACT table during input loads.
"""
import numpy as np

import concourse.bacc as bacc
import concourse.mybir as mybir
import concourse.tile as tile
from concourse import bass_utils

F32 = mybir.dt.float32
F16 = mybir.dt.float16
AF = mybir.ActivationFunctionType

T = 20
D = 340
DP = D + 1               # +1 constant feature row carrying the gate bias
H = 128
E = 32
L = 10
M3 = 3
DCH = [(0, 128), (128, 256), (256, DP)]   # contraction chunks of DP
N_CORES = 8


def build_program(b_loc: int):
    assert b_loc % 256 == 0
    NJ = b_loc // 128
    CW = b_loc // 2               # chain width (<= 512)
    assert CW <= 512
    NCB = 2
    if T == 20:
        TGR = [(0, 1), (1, 2)] + [(t, t + 2) for t in range(2, 14, 2)] \
            + [(14, 17), (17, 20)]
    else:
        TGR = [(0, T)]

    nc = bacc.Bacc("TRN2", target_bir_lowering=False, debug=False)
    xt_d = nc.dram_tensor("xt", (DP, T, b_loc), F16, kind="ExternalInput").ap()
    ident_d = nc.dram_tensor("ident", (128, 128), F16,
                             kind="ExternalInput").ap()
    cnt_d = nc.dram_tensor("cn_t", (E, b_loc), F16, kind="ExternalInput").ap()
    pref2_d = nc.dram_tensor("pref2", (128, NJ, L, M3, 2), F16,
                             kind="ExternalInput").ap()
    pref32_d = nc.dram_tensor("pref_g32", (128, NJ, L, M3), F32,
                              kind="ExternalInput").ap()
    wih_d = nc.dram_tensor("w_ih4", (DP, 4 * H), F16, kind="ExternalInput").ap()
    whh_d = nc.dram_tensor("w_hh_t", (H, 4 * H), F16, kind="ExternalInput").ap()
    fcw_d = nc.dram_tensor("fc_w_t", (E, H), F16, kind="ExternalInput").ap()
    fcb_d = nc.dram_tensor("fc_b_row", (1, H), F16, kind="ExternalInput").ap()
    ones_d = nc.dram_tensor("ones_row", (1, 128), F16, kind="ExternalInput").ap()
    oy1 = nc.dram_tensor("out_y1", (b_loc, L, M3, 128), F16,
                         kind="ExternalOutput").ap()
    oy2 = nc.dram_tensor("out_y2", (b_loc, L, M3, 128), F16,
                         kind="ExternalOutput").ap()

    with tile.TileContext(nc) as tc:
        with tc.tile_pool(name="wpool", bufs=1) as wpool, \
             tc.tile_pool(name="main", bufs=1) as pool, \
             tc.tile_pool(name="psum", bufs=1, space="PSUM") as pspool:

            # ---- weights / constants ----
            ones_t = wpool.tile([1, 128], F16, name="ones_t")
            nc.sync.dma_start(ones_t[:], ones_d)
            # dummy sigmoid: pulls the ACT table load off the critical path
            warm_t = wpool.tile([1, 128], F16, name="warm_t")
            nc.scalar.activation(warm_t[:], ones_t[:], AF.Sigmoid)
            wih_t = []
            for k, (c0, c1) in enumerate(DCH):
                wt_ = wpool.tile([c1 - c0, 4 * H], F16, name=f"wih{k}")
                nc.sync.dma_start(wt_[:], wih_d[c0:c1, :])
                wih_t.append(wt_)
            whh_t = wpool.tile([H, 4 * H], F16, name="whh_t")
            nc.sync.dma_start(whh_t[:], whh_d)
            cnt_t = wpool.tile([E, b_loc], F16, name="cnt_t")
            nc.gpsimd.dma_start(cnt_t[:], cnt_d)
            fcw_t = wpool.tile([E, H], F16, name="fcw_t")
            nc.gpsimd.dma_start(fcw_t[:], fcw_d)
            fcb_t = wpool.tile([1, H], F16, name="fcb_t")
            nc.gpsimd.dma_start(fcb_t[:], fcb_d)

            # ---- persistent fp16 xT tiles, loaded in t-groups ----
            xt_tiles = []
            for k, (c0, c1) in enumerate(DCH):
                xt_tiles.append(
                    pool.tile([c1 - c0, T, b_loc], F16, name=f"xt{k}",
                              tag=f"xt{k}"))
            for (t0, t1) in TGR:
                for k, (c0, c1) in enumerate(DCH):
                    nc.sync.dma_start(xt_tiles[k][:, t0:t1, :],
                                      xt_d[c0:c1, t0:t1, :])

            ident_t = wpool.tile([128, 128], F16, name="ident_t")
            nc.gpsimd.dma_start(ident_t[:], ident_d)
            pf2_t = wpool.tile([128, NJ, L, M3, 2], F16, name="pf2_t")
            nc.gpsimd.dma_start(pf2_t[:], pref2_d)
            pf32_t = wpool.tile([128, NJ, L, M3], F32, name="pf32_t")
            nc.gpsimd.dma_start(pf32_t[:], pref32_d)

            # ---- PSUM per chain, split per ACT reader ----
            Pif, Pg, Po = [], [], []
            for cb in range(NCB):
                Pif.append(pspool.tile([128, 2, 512], F32, name=f"pif{cb}",
                                       tag=f"pif{cb}"))
                Pg.append(pspool.tile([128, 512], F32, name=f"pg{cb}",
                                      tag=f"pg{cb}"))
                Po.append(pspool.tile([128, 512], F32, name=f"po{cb}",
                                      tag=f"po{cb}"))

            def emit_einsum_p2(j, y_half, odram, l0, nl, engine, store_eng):
                """pair-duplicated pref broadcast mul: DVE 2x_1P mode."""
                ol = pool.tile([128, nl, M3, 128], F16, name="ol",
                               tag="outl", bufs=10)
                y_b = y_half[:, None, None, :].rearrange(
                    "p a b (n t) -> p a b n t", t=2).broadcast_to(
                    [128, nl, M3, 64, 2])
                p_b = pf2_t[:, j, l0:l0 + nl, :, None, :].broadcast_to(
                    [128, nl, M3, 64, 2])
                engine.tensor_mul(
                    ol[:].rearrange("p a b (n t) -> p a b n t", t=2),
                    y_b, p_b)
                store_eng.dma_start(
                    odram[j * 128:(j + 1) * 128, l0:l0 + nl, :, :], ol[:])

            def emit_einsum_act(j, y_half, odram, l0, nl, store_eng):
                ol = pool.tile([128, nl, M3, 128], F16, name="ol",
                               tag="outl", bufs=10)
                for li in range(nl):
                    for m in range(M3):
                        sc = pf32_t[:, j, l0 + li, m:m + 1]
                        nc.scalar.mul(ol[:, li, m, :], y_half[:], sc)
                store_eng.dma_start(
                    odram[j * 128:(j + 1) * 128, l0:l0 + nl, :, :], ol[:])

            # ---- y2 head: borrows chain B's Po/Pg banks; emitted after
            # proj(0, A) so chain A's first sigmoid isn't delayed ----
            y2b = []

            def emit_y2_head():
                for jj in range(NJ // 4):
                    bank = (Po[1] if jj == 0 else Pg[1])
                    tgt4 = bank[:, 0:512]
                    for j4 in range(4):
                        j = jj * 4 + j4
                        tgt = bank[:, j4 * 128:(j4 + 1) * 128]
                        nc.tensor.matmul(tgt,
                                         cnt_t[:, j * 128:(j + 1) * 128],
                                         fcw_t[:], start=True, stop=False)
                        nc.tensor.matmul(tgt, ones_t[:], fcb_t[:],
                                         start=False, stop=True)
                    yb = pool.tile([128, 512], F16, name="y2b", tag="y2b",
                                   bufs=max(1, NJ // 4))
                    nc.vector.tensor_scalar_max(yb[:], tgt4, 0.0)
                    y2b.append(yb)

            def y2_src(j):
                return y2b[j // 4][:, (j % 4) * 128:(j % 4) * 128 + 128]

            # y2 einsum jobs: ('v', j, l0, nl) on DVE, ('g', ...) on GpSimd
            y2_jobs = []
            for j in range(NJ):
                if j < 6:
                    y2_jobs.append(('v', j, 0, 5))
                    y2_jobs.append(('v', j, 5, 5))
                else:
                    for l0, nl in ((0, 2), (2, 3), (5, 2), (7, 3)):
                        y2_jobs.append(('g', j, l0, nl))

            def emit_proj(t, cb, stop):
                cs = slice(cb * CW, (cb + 1) * CW)

                def mm(dst, g, k):
                    nc.tensor.matmul(
                        dst, wih_t[k][:, g * 128:(g + 1) * 128],
                        xt_tiles[k][:, t, cs],
                        start=(k == 0), stop=(stop and k == 2))
                for k in range(3):              # i, f pairs first
                    mm(Pif[cb][:, 0, 0:CW], 0, k)
                    mm(Pif[cb][:, 1, 0:CW], 1, k)
                for k in range(3):              # then g (cell)
                    mm(Pg[cb][:, 0:CW], 2, k)
                for k in range(3):              # then o
                    mm(Po[cb][:, 0:CW], 3, k)

            def emit_rec(cb, h_prev):
                for g, dst in ((0, Pif[cb][:, 0, 0:CW]),
                               (1, Pif[cb][:, 1, 0:CW]),
                               (2, Pg[cb][:, 0:CW]),
                               (3, Po[cb][:, 0:CW])):
                    nc.tensor.matmul(dst, whh_t[:, g * 128:(g + 1) * 128],
                                     h_prev[:], start=False, stop=True)

            def new_state(tag):
                return pool.tile([128, CW], F16, name=tag, tag=tag, bufs=2)

            # ---- offset-chain recurrence: chain B trails chain A by OFF
            # steps so chain A's y1 tail hides under chain B's solo steps
            OFF = 1
            NJH = NJ // 2
            state = [{'h': None, 'c': None} for _ in range(NCB)]

            def chain_step(cb, t):
                st = state[cb]
                if t > 0:
                    emit_rec(cb, st['h'])
                gif = pool.tile([128, 2, CW], F16, name="gif",
                                tag=f"gif{cb}", bufs=2)
                gg = new_state(f"gg{cb}")
                go = new_state(f"go{cb}")
                nc.scalar.activation(gif[:], Pif[cb][:, :, 0:CW], AF.Sigmoid)
                nc.scalar.activation(gg[:], Pg[cb][:, 0:CW], AF.Tanh)
                nc.scalar.activation(go[:], Po[cb][:, 0:CW], AF.Sigmoid)
                if t > 0:
                    t1_ = new_state(f"t1{cb}")
                    nc.vector.tensor_mul(t1_[:], gif[:, 1, :], st['c'][:])
                t2_ = new_state(f"t2{cb}")
                nc.vector.tensor_mul(t2_[:], gif[:, 0, :], gg[:])
                if t > 0:
                    c_ = new_state(f"c{cb}")
                    nc.vector.tensor_add(c_[:], t1_[:], t2_[:])
                else:
                    c_ = t2_
                tc_ = new_state(f"tc{cb}")
                h_ = new_state(f"h{cb}")
                nc.scalar.activation(tc_[:], c_[:], AF.Tanh)
                nc.vector.tensor_mul(h_[:], go[:], tc_[:])
                if t + 1 < T:
                    emit_proj(t + 1, cb, stop=False)
                st['h'], st['c'] = h_, c_

            y1b = [None] * NJ

            def emit_transposes(cb):
                # PE transposes into DISTINCT dead PSUM tiles of this chain
                # (tile-granular WAR would serialize slices of one tile);
                # the 4th block goes through the DMA xbar instead.
                dead = [Po[cb][:], Pg[cb][:], Pif[cb][:, 0, :]]
                h_ = state[cb]['h']
                for jj in range(NJH):
                    j = cb * NJH + jj
                    y1 = pool.tile([128, 128], F16, name="y1b", tag="y1b",
                                   bufs=NJ)
                    src = h_[:, jj * 128:(jj + 1) * 128]
                    if jj < min(3, NJH - 1):
                        dst = dead[jj].bitcast(F16)[:, 0:128]
                        nc.tensor.transpose(dst, src, ident_t[:])
                        nc.vector.tensor_copy(y1[:], dst)
                    else:
                        nc.sync.dma_start_transpose(y1[:], src)
                    y1b[j] = y1

            def emit_y1_job(idx, j, l0, kind):
                st_q = nc.sync if idx % 2 == 0 else nc.gpsimd
                if kind == 'a':
                    emit_einsum_act(j, y1b[j], oy1, l0, 5, st_q)
                elif kind == 'g':
                    emit_einsum_p2(j, y1b[j], oy1, l0, 5, nc.gpsimd, st_q)
                else:
                    emit_einsum_p2(j, y1b[j], oy1, l0, 5, nc.vector, st_q)

            a_jobs = [(j, 0) for j in range(NJH)] \
                + [(j, 5) for j in range(NJH)]
            b_jobs = [(NJH + j, 0) for j in range(NJH)] \
                + [(NJH + j, 5) for j in range(NJH)]

            # ---- prologue projections for t=0 (both chains); y2 head
            # in between, borrowing chain B's PSUM banks ----
            emit_proj(0, 0, stop=True)
            emit_y2_head()
            emit_proj(0, 1, stop=True)

            for s in range(T + OFF):
                if s < T:
                    chain_step(0, s)
                if s >= OFF:
                    chain_step(1, s - OFF)
                if s == T - 1:
                    emit_transposes(0)      # chain A tail setup

                # chain A y1 einsum jobs hide under chain B's solo steps
                if s == T:
                    for i, (j, l0) in enumerate(a_jobs):
                        emit_y1_job(i, j, l0, 'a' if i == 2 else 'v')

                # y2 einsum jobs ride the dual-chain recurrence
                if 2 <= s < T and y2_jobs:
                    take, nv, ng = [], 0, 0
                    for job in y2_jobs:
                        if job[0] == 'v' and nv < 1:
                            take.append(job); nv += 1
                        elif job[0] == 'g' and ng < 1:
                            take.append(job); ng += 1
                    for job in take:
                        y2_jobs.remove(job)
                    for kind, j, l0, nl in take:
                        eng = nc.vector if kind == 'v' else nc.gpsimd
                        emit_einsum_p2(j, y2_src(j), oy2, l0, nl,
                                       eng, nc.gpsimd)

            # ---- chain B tail ----
            emit_transposes(1)
            for i, (j, l0) in enumerate(b_jobs, len(a_jobs)):
                kind = 'a' if i == len(a_jobs) else 'v'
                emit_y1_job(i, j, l0, kind)

    nc.compile()
    return nc


def prep_in_maps(inputs, n_cores: int, b_loc: int):
    history = np.asarray(inputs["history"], np.float32)
    cluster = np.asarray(inputs["cluster_num"], np.float32)
    pref = np.asarray(inputs["pref"], np.float32)
    w_ih = np.asarray(inputs["W_ih"], np.float32)
    w_hh = np.asarray(inputs["W_hh"], np.float32)
    b_ih = np.asarray(inputs["b_ih"], np.float32)
    b_hh = np.asarray(inputs["b_hh"], np.float32)
    fc_w = np.asarray(inputs["fc_w"], np.float32)
    fc_b = np.asarray(inputs["fc_b"], np.float32)

    NJ = b_loc // 128
    w_ih4 = np.concatenate(
        [w_ih.T, (b_ih + b_hh).reshape(1, 4 * H)], axis=0)  # [341, 512]
    shared = {
        "w_ih4": np.ascontiguousarray(w_ih4.astype(np.float16)),
        "w_hh_t": np.ascontiguousarray(w_hh.T.astype(np.float16)),
        "fc_w_t": np.ascontiguousarray(fc_w.T.astype(np.float16)),
        "fc_b_row": np.ascontiguousarray(fc_b.reshape(1, H).astype(np.float16)),
        "ones_row": np.ones((1, 128), np.float16),
    }
    in_maps = []
    for c in range(n_cores):
        r0, r1 = c * b_loc, (c + 1) * b_loc
        hist16 = history[r0:r1].reshape(b_loc, T, D).astype(np.float16)
        xt = np.empty((DP, T, b_loc), np.float16)
        xt[:D] = hist16.transpose(2, 1, 0)
        xt[D] = 1.0
        shared["ident"] = np.eye(128, dtype=np.float16)
        pref16 = pref[r0:r1].reshape(NJ, 128, L, M3).astype(np.float16)
        pg = np.ascontiguousarray(pref16.transpose(1, 0, 2, 3))
        in_maps.append({
            "xt": xt,
            "cn_t": np.ascontiguousarray(
                cluster[r0:r1].T.astype(np.float16)),
            "pref2": np.ascontiguousarray(
                np.repeat(pg[..., None], 2, axis=-1)),
            "pref_g32": pg.astype(np.float32),
            **shared,
        })
    return in_maps


def run(inputs, n_cores: int = N_CORES, trace: bool = False):
    B = np.asarray(inputs["history"]).shape[0]
    b_loc = B // n_cores
    nc = build_program(b_loc)
    in_maps = prep_in_maps(inputs, n_cores, b_loc)
    res = bass_utils.run_bass_kernel_spmd(
        nc, in_maps, core_ids=list(range(n_cores)), trace=trace)
    outs = []
    for c in range(n_cores):
        y1 = res.results[c]["out_y1"].astype(np.float32)
        y2 = res.results[c]["out_y2"].astype(np.float32)
        o = np.concatenate([y1, y2], axis=3)         # [b, L, M3, 256]
        outs.append(o.transpose(0, 1, 3, 2).reshape(b_loc, L, 256 * M3))
    return np.concatenate(outs, axis=0), res


def kernel(**inputs) -> np.ndarray:
    out, _ = run(inputs, N_CORES)
    return out


# revision 39
# speedup vs baseline: 1.0015x; 1.0015x over previous
"""Trainium2 Bass kernel for nn_BackBone (LSTM backbone + fc + outer-product head).

Data-parallel over batch across 8 NeuronCores. Per core (b_loc rows), v9:
  - history transposed + cast to fp16 on the HOST: xt[D+1, T, B] with a
    constant-1.0 feature row appended so the gate bias rides the projection
    matmul. All matmuls fp16 (fp8 DoubleRow measured 424ns per 512-col MM =
    exactly two fp16 matmuls; no streaming win, so fp16 keeps accuracy).
  - two 512-col batch chains with chain B OFFSET one step behind chain A,
    per-READER gate PSUMs (Pif merged sigmoid(i,f), Pg tanh, Po sigmoid) so
    proj(t+1) matmuls only wait on their own gate's activation read ->
    near-zero-stall PE steady state at ~215ns/MM (the binding engine:
    32 N=512 matmuls/step = 6.9us vs ACT 6.5us).
  - head einsum: broadcast tensor_tensor with PAIR-DUPLICATED pref
    (pf2[...,2]); the step-1 inner fp16 pair unlocks DVE 2x_1P mode: 1.15us
    per [128,5,3,128] job vs 2.15us classic. y2 jobs ride the recurrence on
    DVE + small GpSimd bites (no ACT einsum while ACT paces the LSTM; GpSimd
    and DVE share an SBUF port, so concurrent big jobs on both mutually
    stall - GpSimd gets only small early bites).
  - y1 tail: chain A finishes one step early; its h transposes run on the PE
    (three into distinct DEAD gate-PSUM banks via fp16 bitcast - distinct
    tiles because Tile WAR tracking is tile-granular - plus one DMA-xbar)
    and its 8 einsum jobs hide under chain B's final solo step. Only chain
    B's short tail (PE transposes + 7 DVE jobs + 1 ACT job) is exposed.
  - y2 head borrows chain B's Po/Pg PSUM banks between the two t=0
    projection groups; relu on DVE (tensor_scalar_max) keeps the single ACT
    table-set; a dummy sigmoid preloads the ACT table during input loads.
"""
import numpy as np

import concourse.bacc as bacc
import concourse.mybir as mybir
import concourse.tile as tile
from concourse import bass_utils

F32 = mybir.dt.float32
F16 = mybir.dt.float16
AF = mybir.ActivationFunctionType

T = 20
D = 340
DP = D + 1               # +1 constant feature row carrying the gate bias
H = 128
E = 32
L = 10
M3 = 3
DCH = [(0, 128), (128, 256), (256, DP)]   # contraction chunks of DP
N_CORES = 8


def build_program(b_loc: int):
    assert b_loc % 256 == 0
    NJ = b_loc // 128
    CW = b_loc // 2               # chain width (<= 512)
    assert CW <= 512
    NCB = 2
    if T == 20:
        TGR = [(0, 1), (1, 2)] + [(t, t + 2) for t in range(2, 14, 2)] \
            + [(14, 17), (17, 20)]
    else:
        TGR = [(0, T)]

    nc = bacc.Bacc("TRN2", target_bir_lowering=False, debug=False)
    xt_d = nc.dram_tensor("xt", (DP, T, b_loc), F16, kind="ExternalInput").ap()
    ident_d = nc.dram_tensor("ident", (128, 128), F16,
                             kind="ExternalInput").ap()
    cnt_d = nc.dram_tensor("cn_t", (E, b_loc), F16, kind="ExternalInput").ap()
    pref2_d = nc.dram_tensor("pref2", (128, NJ, L, M3, 2), F16,
                             kind="ExternalInput").ap()
    pref32_d = nc.dram_tensor("pref_g32", (128, NJ, L, M3), F32,
                              kind="ExternalInput").ap()
    wih_d = nc.dram_tensor("w_ih4", (DP, 4 * H), F16, kind="ExternalInput").ap()
    whh_d = nc.dram_tensor("w_hh_t", (H, 4 * H), F16, kind="ExternalInput").ap()
    fcw_d = nc.dram_tensor("fc_w_t", (E, H), F16, kind="ExternalInput").ap()
    fcb_d = nc.dram_tensor("fc_b_row", (1, H), F16, kind="ExternalInput").ap()
    ones_d = nc.dram_tensor("ones_row", (1, 128), F16, kind="ExternalInput").ap()
    oy1 = nc.dram_tensor("out_y1", (b_loc, L, M3, 128), F16,
                         kind="ExternalOutput").ap()
    oy2 = nc.dram_tensor("out_y2", (b_loc, L, M3, 128), F16,
                         kind="ExternalOutput").ap()

    with tile.TileContext(nc) as tc:
        with tc.tile_pool(name="wpool", bufs=1) as wpool, \
             tc.tile_pool(name="main", bufs=1) as pool, \
             tc.tile_pool(name="psum", bufs=1, space="PSUM") as pspool:

            # ---- weights / constants ----
            ones_t = wpool.tile([1, 128], F16, name="ones_t")
            nc.sync.dma_start(ones_t[:], ones_d)
            # dummy sigmoid: pulls the ACT table load off the critical path
            warm_t = wpool.tile([1, 128], F16, name="warm_t")
            nc.scalar.activation(warm_t[:], ones_t[:], AF.Sigmoid)
            wih_t = []
            for k, (c0, c1) in enumerate(DCH):
                wt_ = wpool.tile([c1 - c0, 4 * H], F16, name=f"wih{k}")
                nc.sync.dma_start(wt_[:], wih_d[c0:c1, :])
                wih_t.append(wt_)
            whh_t = wpool.tile([H, 4 * H], F16, name="whh_t")
            nc.sync.dma_start(whh_t[:], whh_d)
            cnt_t = wpool.tile([E, b_loc], F16, name="cnt_t")
            nc.gpsimd.dma_start(cnt_t[:], cnt_d)
            fcw_t = wpool.tile([E, H], F16, name="fcw_t")
            nc.gpsimd.dma_start(fcw_t[:], fcw_d)
            fcb_t = wpool.tile([1, H], F16, name="fcb_t")
            nc.gpsimd.dma_start(fcb_t[:], fcb_d)

            # ---- persistent fp16 xT tiles, loaded in t-groups ----
            xt_tiles = []
            for k, (c0, c1) in enumerate(DCH):
                xt_tiles.append(
                    pool.tile([c1 - c0, T, b_loc], F16, name=f"xt{k}",
                              tag=f"xt{k}"))
            for gi, (t0, t1) in enumerate(TGR):
                if gi < 2:
                    # chain A's columns first: its projection starts sooner
                    for k, (c0, c1) in enumerate(DCH):
                        nc.sync.dma_start(xt_tiles[k][:, t0:t1, 0:CW],
                                          xt_d[c0:c1, t0:t1, 0:CW])
                    for k, (c0, c1) in enumerate(DCH):
                        nc.sync.dma_start(xt_tiles[k][:, t0:t1, CW:],
                                          xt_d[c0:c1, t0:t1, CW:])
                else:
                    for k, (c0, c1) in enumerate(DCH):
                        nc.sync.dma_start(xt_tiles[k][:, t0:t1, :],
                                          xt_d[c0:c1, t0:t1, :])

            ident_t = wpool.tile([128, 128], F16, name="ident_t")
            nc.gpsimd.dma_start(ident_t[:], ident_d)
            pf2_t = wpool.tile([128, NJ, L, M3, 2], F16, name="pf2_t")
            nc.gpsimd.dma_start(pf2_t[:], pref2_d)
            pf32_t = wpool.tile([128, NJ, L, M3], F32, name="pf32_t")
            nc.gpsimd.dma_start(pf32_t[:], pref32_d)

            # ---- PSUM per chain, split per ACT reader ----
            Pif, Pg, Po = [], [], []
            for cb in range(NCB):
                Pif.append(pspool.tile([128, 2, 512], F32, name=f"pif{cb}",
                                       tag=f"pif{cb}"))
                Pg.append(pspool.tile([128, 512], F32, name=f"pg{cb}",
                                      tag=f"pg{cb}"))
                Po.append(pspool.tile([128, 512], F32, name=f"po{cb}",
                                      tag=f"po{cb}"))

            def emit_einsum_p2(j, y_half, odram, l0, nl, engine, store_eng):
                """pair-duplicated pref broadcast mul: DVE 2x_1P mode."""
                ol = pool.tile([128, nl, M3, 128], F16, name="ol",
                               tag="outl", bufs=10)
                y_b = y_half[:, None, None, :].rearrange(
                    "p a b (n t) -> p a b n t", t=2).broadcast_to(
                    [128, nl, M3, 64, 2])
                p_b = pf2_t[:, j, l0:l0 + nl, :, None, :].broadcast_to(
                    [128, nl, M3, 64, 2])
                engine.tensor_mul(
                    ol[:].rearrange("p a b (n t) -> p a b n t", t=2),
                    y_b, p_b)
                store_eng.dma_start(
                    odram[j * 128:(j + 1) * 128, l0:l0 + nl, :, :], ol[:])

            def emit_einsum_act(j, y_half, odram, l0, nl, store_eng):
                ol = pool.tile([128, nl, M3, 128], F16, name="ol",
                               tag="outl", bufs=10)
                for li in range(nl):
                    for m in range(M3):
                        sc = pf32_t[:, j, l0 + li, m:m + 1]
                        nc.scalar.mul(ol[:, li, m, :], y_half[:], sc)
                store_eng.dma_start(
                    odram[j * 128:(j + 1) * 128, l0:l0 + nl, :, :], ol[:])

            # ---- y2 head: borrows chain B's Po/Pg banks; emitted after
            # proj(0, A) so chain A's first sigmoid isn't delayed ----
            y2b = []

            def emit_y2_head():
                for jj in range(NJ // 4):
                    bank = (Po[1] if jj == 0 else Pg[1])
                    tgt4 = bank[:, 0:512]
                    for j4 in range(4):
                        j = jj * 4 + j4
                        tgt = bank[:, j4 * 128:(j4 + 1) * 128]
                        nc.tensor.matmul(tgt,
                                         cnt_t[:, j * 128:(j + 1) * 128],
                                         fcw_t[:], start=True, stop=False)
                        nc.tensor.matmul(tgt, ones_t[:], fcb_t[:],
                                         start=False, stop=True)
                    yb = pool.tile([128, 512], F16, name="y2b", tag="y2b",
                                   bufs=max(1, NJ // 4))
                    nc.vector.tensor_scalar_max(yb[:], tgt4, 0.0)
                    y2b.append(yb)

            def y2_src(j):
                return y2b[j // 4][:, (j % 4) * 128:(j % 4) * 128 + 128]

            # y2 einsum jobs: ('v', j, l0, nl) on DVE, ('g', ...) on GpSimd
            y2_jobs = []
            for j in range(NJ):
                if j < 6:
                    y2_jobs.append(('v', j, 0, 5))
                    y2_jobs.append(('v', j, 5, 5))
                else:
                    for l0, nl in ((0, 2), (2, 3), (5, 2), (7, 3)):
                        y2_jobs.append(('g', j, l0, nl))

            def emit_proj(t, cb, stop):
                cs = slice(cb * CW, (cb + 1) * CW)

                def mm(dst, g, k):
                    nc.tensor.matmul(
                        dst, wih_t[k][:, g * 128:(g + 1) * 128],
                        xt_tiles[k][:, t, cs],
                        start=(k == 0), stop=(stop and k == 2))
                for k in range(3):              # i, f pairs first
                    mm(Pif[cb][:, 0, 0:CW], 0, k)
                    mm(Pif[cb][:, 1, 0:CW], 1, k)
                for k in range(3):              # then g (cell)
                    mm(Pg[cb][:, 0:CW], 2, k)
                for k in range(3):              # then o
                    mm(Po[cb][:, 0:CW], 3, k)

            def emit_rec(cb, h_prev):
                for g, dst in ((0, Pif[cb][:, 0, 0:CW]),
                               (1, Pif[cb][:, 1, 0:CW]),
                               (2, Pg[cb][:, 0:CW]),
                               (3, Po[cb][:, 0:CW])):
                    nc.tensor.matmul(dst, whh_t[:, g * 128:(g + 1) * 128],
                                     h_prev[:], start=False, stop=True)

            def new_state(tag):
                return pool.tile([128, CW], F16, name=tag, tag=tag, bufs=2)

            # ---- offset-chain recurrence: chain B trails chain A by OFF
            # steps so chain A's y1 tail hides under chain B's solo steps
            OFF = 1
            NJH = NJ // 2
            state = [{'h': None, 'c': None} for _ in range(NCB)]

            def chain_step(cb, t):
                st = state[cb]
                if t > 0:
                    emit_rec(cb, st['h'])
                gif = pool.tile([128, 2, CW], F16, name="gif",
                                tag=f"gif{cb}", bufs=2)
                gg = new_state(f"gg{cb}")
                go = new_state(f"go{cb}")
                nc.scalar.activation(gif[:], Pif[cb][:, :, 0:CW], AF.Sigmoid)
                nc.scalar.activation(gg[:], Pg[cb][:, 0:CW], AF.Tanh)
                nc.scalar.activation(go[:], Po[cb][:, 0:CW], AF.Sigmoid)
                if t > 0:
                    t1_ = new_state(f"t1{cb}")
                    nc.vector.tensor_mul(t1_[:], gif[:, 1, :], st['c'][:])
                t2_ = new_state(f"t2{cb}")
                nc.vector.tensor_mul(t2_[:], gif[:, 0, :], gg[:])
                if t > 0:
                    c_ = new_state(f"c{cb}")
                    nc.vector.tensor_add(c_[:], t1_[:], t2_[:])
                else:
                    c_ = t2_
                tc_ = new_state(f"tc{cb}")
                h_ = new_state(f"h{cb}")
                nc.scalar.activation(tc_[:], c_[:], AF.Tanh)
                nc.vector.tensor_mul(h_[:], go[:], tc_[:])
                if t + 1 < T:
                    emit_proj(t + 1, cb, stop=False)
                st['h'], st['c'] = h_, c_

            y1b = [None] * NJ

            def emit_transposes(cb):
                # PE transposes into DISTINCT dead PSUM tiles of this chain
                # (tile-granular WAR would serialize slices of one tile);
                # the 4th block goes through the DMA xbar instead.
                dead = [Po[cb][:], Pg[cb][:], Pif[cb][:, 0, :]]
                h_ = state[cb]['h']
                for jj in range(NJH):
                    j = cb * NJH + jj
                    y1 = pool.tile([128, 128], F16, name="y1b", tag="y1b",
                                   bufs=NJ)
                    src = h_[:, jj * 128:(jj + 1) * 128]
                    if jj < min(3, NJH - 1):
                        dst = dead[jj].bitcast(F16)[:, 0:128]
                        nc.tensor.transpose(dst, src, ident_t[:])
                        nc.vector.tensor_copy(y1[:], dst)
                    else:
                        nc.sync.dma_start_transpose(y1[:], src)
                    y1b[j] = y1

            def emit_y1_job(idx, j, l0, kind):
                st_q = nc.sync if idx % 2 == 0 else nc.gpsimd
                if kind == 'a':
                    emit_einsum_act(j, y1b[j], oy1, l0, 5, st_q)
                elif kind == 'g':
                    emit_einsum_p2(j, y1b[j], oy1, l0, 5, nc.gpsimd, st_q)
                else:
                    emit_einsum_p2(j, y1b[j], oy1, l0, 5, nc.vector, st_q)

            a_jobs = [(j, 0) for j in range(NJH)] \
                + [(j, 5) for j in range(NJH)]
            b_jobs = [(NJH + j, 0) for j in range(NJH)] \
                + [(NJH + j, 5) for j in range(NJH)]

            # ---- prologue projections for t=0 (both chains); y2 head
            # in between, borrowing chain B's PSUM banks ----
            emit_proj(0, 0, stop=True)
            emit_y2_head()
            emit_proj(0, 1, stop=True)

            for s in range(T + OFF):
                if s < T:
                    chain_step(0, s)
                if s >= OFF:
                    chain_step(1, s - OFF)
                if s == T - 1:
                    emit_transposes(0)      # chain A tail setup

                # chain A y1 einsum jobs hide under chain B's solo steps
                if s == T:
                    for i, (j, l0) in enumerate(a_jobs):
                        emit_y1_job(i, j, l0, 'a' if i == 2 else 'v')

                # y2 einsum jobs ride the dual-chain recurrence
                if 2 <= s < T and y2_jobs:
                    take, nv, ng = [], 0, 0
                    for job in y2_jobs:
                        if job[0] == 'v' and nv < 1:
                            take.append(job); nv += 1
                        elif job[0] == 'g' and ng < 1:
                            take.append(job); ng += 1
                    for job in take:
                        y2_jobs.remove(job)
                    for kind, j, l0, nl in take:
                        eng = nc.vector if kind == 'v' else nc.gpsimd
                        emit_einsum_p2(j, y2_src(j), oy2, l0, nl,
                                       eng, nc.gpsimd)

            # ---- chain B tail ----
            emit_transposes(1)
            for i, (j, l0) in enumerate(b_jobs, len(a_jobs)):
                kind = 'a' if i == len(a_jobs) else 'v'
                emit_y1_job(i, j, l0, kind)

    nc.compile()
    return nc


def prep_in_maps(inputs, n_cores: int, b_loc: int):
    history = np.asarray(inputs["history"], np.float32)
    cluster = np.asarray(inputs["cluster_num"], np.float32)
    pref = np.asarray(inputs["pref"], np.float32)
    w_ih = np.asarray(inputs["W_ih"], np.float32)
    w_hh = np.asarray(inputs["W_hh"], np.float32)
    b_ih = np.asarray(inputs["b_ih"], np.float32)
    b_hh = np.asarray(inputs["b_hh"], np.float32)
    fc_w = np.asarray(inputs["fc_w"], np.float32)
    fc_b = np.asarray(inputs["fc_b"], np.float32)

    NJ = b_loc // 128
    w_ih4 = np.concatenate(
        [w_ih.T, (b_ih + b_hh).reshape(1, 4 * H)], axis=0)  # [341, 512]
    shared = {
        "w_ih4": np.ascontiguousarray(w_ih4.astype(np.float16)),
        "w_hh_t": np.ascontiguousarray(w_hh.T.astype(np.float16)),
        "fc_w_t": np.ascontiguousarray(fc_w.T.astype(np.float16)),
        "fc_b_row": np.ascontiguousarray(fc_b.reshape(1, H).astype(np.float16)),
        "ones_row": np.ones((1, 128), np.float16),
    }
    in_maps = []
    for c in range(n_cores):
        r0, r1 = c * b_loc, (c + 1) * b_loc
        hist16 = history[r0:r1].reshape(b_loc, T, D).astype(np.float16)
        xt = np.empty((DP, T, b_loc), np.float16)
        xt[:D] = hist16.transpose(2, 1, 0)
        xt[D] = 1.0
        shared["ident"] = np.eye(128, dtype=np.float16)
        pref16 = pref[r0:r1].reshape(NJ, 128, L, M3).astype(np.float16)
        pg = np.ascontiguousarray(pref16.transpose(1, 0, 2, 3))
        in_maps.append({
            "xt": xt,
            "cn_t": np.ascontiguousarray(
                cluster[r0:r1].T.astype(np.float16)),
            "pref2": np.ascontiguousarray(
                np.repeat(pg[..., None], 2, axis=-1)),
            "pref_g32": pg.astype(np.float32),
            **shared,
        })
    return in_maps


def run(inputs, n_cores: int = N_CORES, trace: bool = False):
    B = np.asarray(inputs["history"]).shape[0]
    b_loc = B // n_cores
    nc = build_program(b_loc)
    in_maps = prep_in_maps(inputs, n_cores, b_loc)
    res = bass_utils.run_bass_kernel_spmd(
        nc, in_maps, core_ids=list(range(n_cores)), trace=trace)
    outs = []
    for c in range(n_cores):
        y1 = res.results[c]["out_y1"].astype(np.float32)
        y2 = res.results[c]["out_y2"].astype(np.float32)
        o = np.concatenate([y1, y2], axis=3)         # [b, L, M3, 256]
        outs.append(o.transpose(0, 1, 3, 2).reshape(b_loc, L, 256 * M3))
    return np.concatenate(outs, axis=0), res


def kernel(**inputs) -> np.ndarray:
    out, _ = run(inputs, N_CORES)
    return out
